# revision 21
# baseline (speedup 1.0000x reference)
"""Trainium2 Bass kernel for the DRSAR-FL fuzzy-gated RNN.

Model (per reference):
  weights = sigmoid(theta);  xf = x * weights
  xw = einsum('btf,fh->bth', xf, Wx);  new_info = tanh(xw)
  scan over t:  z = xw_t + h @ Wh + b
                mu_k = exp(-(z-c_k)^2 * inv_k)            (K=3)
                g = sigmoid(sum(mu q)/ (sum(mu)+1e-8))
                h = (1-g) h + g new_info_t
  logits = h @ Wc + bc;  returns (logits, weights)

Sharding: data-parallel over batch B=512 -> 8 cores x 64. Weights replicated.

On-chip layout ("transposed-chunk merged"): the hidden state h[64,512] is kept
transposed as hT[128, 4*64] where partition = h%128, free = (hchunk, b).  This
makes the recurrent matmul a 16x [128k,128m]x[128,64] accumulation (contraction
on partitions) and keeps every elementwise op a full-width [128,256] tile.
xw is precomputed as xw_scan[128, T, 4, 64] so each scan step reads a dense
[128,256] slice.  x is transposed on-chip via PE-transpose; the xw matmul runs
in float32r (full-rate fp32 PE path).
"""

import numpy as np
from contextlib import ExitStack

import concourse.bacc as bacc
import concourse.bass as bass
import concourse.mybir as mybir
import concourse.tile as tile
from concourse import bass_utils

F32 = mybir.dt.float32
F32R = mybir.dt.float32r
BF16 = mybir.dt.bfloat16
AF = mybir.ActivationFunctionType
OP = mybir.AluOpType

B, T, F, H, K = 512, 128, 256, 512, 3
NCORES = 8
BL = B // NCORES          # 64 batch per core
NC_H = H // 128           # 4 h-chunks
EPS = 1e-8

# knobs
MM_BF16 = True        # recurrent h@Wh matmul in bf16
USE_DIVIDE = False    # DVE tensor_tensor divide is not a valid ISA op on TRN2
POOL_OFFLOAD = True   # run some elementwise ops on GpSimd
WARM_PE = 0           # filler matmuls hurt (serialize on PE) - keep off


def _build(nc, neg_inv_imm=None):
    mmdt = BF16 if MM_BF16 else F32
    x = nc.dram_tensor("x", [BL, T, F], F32, kind="ExternalInput")
    wxe = nc.dram_tensor("wxe", [F, H], F32, kind="ExternalInput")
    wh = nc.dram_tensor("wh", [H, H], mmdt, kind="ExternalInput")
    cmu = nc.dram_tensor("cmu", [K, 128, 256], F32, kind="ExternalInput")
    if neg_inv_imm is None:
        negi = nc.dram_tensor("negi", [K, 128, 256], F32, kind="ExternalInput")
    qq = nc.dram_tensor("qq", [K, 128, 256], F32, kind="ExternalInput")
    bb = nc.dram_tensor("bb", [128, NC_H], F32, kind="ExternalInput")
    wc = nc.dram_tensor("wc", [128, NC_H], F32, kind="ExternalInput")
    bcb = nc.dram_tensor("bcb", [1, 1], F32, kind="ExternalInput")
    ident = nc.dram_tensor("ident", [128, 128], F32, kind="ExternalInput")
    louts = nc.dram_tensor("logits", [1, BL], F32, kind="ExternalOutput")

    with tile.TileContext(nc) as tc, ExitStack() as ctx:
        const = ctx.enter_context(tc.tile_pool(name="const", bufs=1))
        wxe_sb = const.tile([128, 2, H], F32, tag="wxe")
        wh_sb = const.tile([128, NC_H, H], mmdt, tag="wh")
        cmu_sb = const.tile([128, K, 256], F32, tag="cmu")
        if neg_inv_imm is None:
            negi_sb = const.tile([128, K, 256], F32, tag="negi")
        qq_sb = const.tile([128, K, 256], F32, tag="qq")
        bb_sb = const.tile([128, NC_H], F32, tag="bb")
        wc_sb = const.tile([128, NC_H], F32, tag="wc")
        bcb_sb = const.tile([1, 1], F32, tag="bcb")
        id_sb = const.tile([128, 128], F32, tag="ident")

        nc.sync.dma_start(wxe_sb[:], wxe.rearrange("(fc p) h -> p fc h", p=128))
        nc.sync.dma_start(wh_sb[:], wh.rearrange("(kc p) h -> p kc h", p=128))
        nc.sync.dma_start(cmu_sb[:], cmu.rearrange("k p j -> p k j"))
        if neg_inv_imm is None:
            nc.sync.dma_start(negi_sb[:], negi.rearrange("k p j -> p k j"))
        nc.sync.dma_start(qq_sb[:], qq.rearrange("k p j -> p k j"))
        nc.sync.dma_start(bb_sb[:], bb[:])
        nc.sync.dma_start(wc_sb[:], wc[:])
        nc.sync.dma_start(bcb_sb[:], bcb[:])
        nc.sync.dma_start(id_sb[:], ident[:])

        xwp = ctx.enter_context(tc.tile_pool(name="xwp", bufs=1))
        xw = xwp.tile([128, T, NC_H, BL], F32, tag="xw")  # [p, t, hc, b]
        # float32r copy of Wx_eff: full-rate fp32 PE path needs f32r-rounded
        # producers for both matmul operands
        wxr = const.tile([128, 2, H], F32R, tag="wxr")
        nc.vector.tensor_copy(wxr[:], wxe_sb[:])

        # ---- precompute: transpose x, xw = xT @ Wx_eff (+b), scan layout ----
        with tc.tile_pool(name="xb", bufs=4) as xbp, \
             tc.tile_pool(name="xt", bufs=2) as xtp, \
             tc.tile_pool(name="ptr", bufs=2, space="PSUM") as ptrp, \
             tc.tile_pool(name="pxw", bufs=4, space="PSUM") as pxwp:
            for g in range(BL // 4):
                xt = xtp.tile([128, 2, 512], F32R, tag="xt")  # [f%128, fc, 4b x t]
                for bi in range(4):
                    xb = xbp.tile([128, 256], F32, tag="xb")
                    nc.sync.dma_start(xb[:], x[g * 4 + bi])
                    for fc in range(2):
                        pt = ptrp.tile([128, 128], F32, tag="pt")
                        nc.tensor.transpose(pt[:], xb[:, fc * 128:(fc + 1) * 128], id_sb[:])
                        dst = xt[:, fc, bi * 128:(bi + 1) * 128]
                        nc.vector.tensor_copy(dst, pt[:])
                for hc in range(NC_H):
                    pxw = pxwp.tile([128, 512], F32, tag="pxw")
                    for fc in range(2):
                        nc.tensor.matmul(
                            pxw[:],
                            wxr[:, fc, hc * 128:(hc + 1) * 128],
                            xt[:, fc, :],
                            start=(fc == 0), stop=(fc == 1),
                        )
                    # pxw free order is (bi, t); write into xw[p, t, hc, g*4+bi]
                    dst = xw[:, :, hc, g * 4:g * 4 + 4].rearrange("p t b -> p b t")
                    src = pxw[:].rearrange("p (b t) -> p b t", b=4)
                    nc.scalar.activation(dst, src, AF.Identity, bias=bb_sb[:, hc:hc + 1])

        # ---- scan over T ----
        with tc.tile_pool(name="psz", bufs=2, space="PSUM") as pszp, \
             tc.tile_pool(name="wk", bufs=2) as wk, \
             tc.tile_pool(name="hp", bufs=2) as hp, \
             tc.tile_pool(name="pl", bufs=1, space="PSUM") as plp, \
             tc.tile_pool(name="warm", bufs=2, space="PSUM") as warmp:
            pool_eng = nc.gpsimd if POOL_OFFLOAD else nc.vector
            h = hp.tile([128, NC_H * BL], F32, tag="h")
            nc.vector.memset(h[:], 0.0)
            hb = hp.tile([128, NC_H * BL], mmdt, tag="hb")
            nc.vector.memset(hb[:], 0.0)
            for t in range(T):
                if WARM_PE and MM_BF16:
                    # dependency-free junk matmuls: keep the PE HAM clock at
                    # 2.4 GHz across the per-step DVE/ACT phases
                    pwarm = warmp.tile([128, 512], F32, tag="pwarm")
                    for w in range(WARM_PE):
                        nc.tensor.matmul(pwarm[:], wh_sb[:, 0, 0:128],
                                         wh_sb[:, 1, :], start=(w == 0),
                                         stop=(w == WARM_PE - 1))
                psz = pszp.tile([128, NC_H * BL], F32, tag="psz")
                for mc in range(NC_H):
                    for kc in range(NC_H):
                        nc.tensor.matmul(
                            psz[:, mc * BL:(mc + 1) * BL],
                            wh_sb[:, kc, mc * 128:(mc + 1) * 128],
                            hb[:, kc * BL:(kc + 1) * BL],
                            start=(kc == 0), stop=(kc == NC_H - 1),
                        )
                xwt = xw[:, t].rearrange("p c b -> p (c b)")
                z = wk.tile([128, 256], F32, tag="z")
                nc.vector.tensor_tensor(z[:], psz[:], xwt, OP.add)
                mus = []
                for k in range(K):
                    d = wk.tile([128, 256], F32, tag=f"d{k}")
                    nc.vector.tensor_tensor(d[:], z[:], cmu_sb[:, k], OP.subtract)
                    mu = wk.tile([128, 256], F32, tag=f"mu{k}")
                    if neg_inv_imm is not None:
                        e = wk.tile([128, 256], F32, tag=f"e{k}")
                        nc.vector.tensor_tensor(e[:], d[:], d[:], OP.mult)
                        nc.scalar.activation(mu[:], e[:], AF.Exp, scale=float(neg_inv_imm))
                    else:
                        u = wk.tile([128, 256], F32, tag=f"u{k}")
                        nc.vector.tensor_tensor(u[:], d[:], negi_sb[:, k], OP.mult)
                        e = wk.tile([128, 256], F32, tag=f"e{k}")
                        nc.vector.tensor_tensor(e[:], u[:], d[:], OP.mult)
                        nc.scalar.activation(mu[:], e[:], AF.Exp)
                    mus.append(mu)
                n = wk.tile([128, 256], F32, tag="n")
                nc.vector.tensor_tensor(n[:], mus[0][:], qq_sb[:, 0], OP.mult)
                t1 = wk.tile([128, 256], F32, tag="t1")
                pool_eng.tensor_tensor(t1[:], mus[1][:], qq_sb[:, 1], OP.mult)
                t2 = wk.tile([128, 256], F32, tag="t2")
                pool_eng.tensor_tensor(t2[:], mus[2][:], qq_sb[:, 2], OP.mult)
                nc.vector.tensor_tensor(n[:], n[:], t1[:], OP.add)
                nc.vector.tensor_tensor(n[:], n[:], t2[:], OP.add)
                dn = wk.tile([128, 256], F32, tag="dn")
                pool_eng.tensor_tensor(dn[:], mus[0][:], mus[1][:], OP.add)
                nc.vector.scalar_tensor_tensor(dn[:], mus[2][:], EPS, dn[:], OP.add, OP.add)
                u = wk.tile([128, 256], F32, tag="u")
                nc.vector.reciprocal_approx_fast(u[:], dn[:])
                nc.vector.tensor_tensor(u[:], n[:], u[:], OP.mult)
                # sigmoid(u) = (1 + tanh(u/2)) / 2 = 0.5*tanh(u/2) + 0.5
                tau = wk.tile([128, 256], F32, tag="tau")
                nc.scalar.activation(tau[:], u[:], AF.Tanh, scale=0.5)
                nt = wk.tile([128, 256], F32, tag="nt")
                nc.scalar.activation(nt[:], xwt, AF.Tanh)
                dlt = wk.tile([128, 256], F32, tag="dlt")
                nc.vector.tensor_tensor(dlt[:], nt[:], h[:], OP.subtract)
                # m = g*(nt-h) with g = 0.5*tau + 0.5, fused into one DVE op
                m = wk.tile([128, 256], F32, tag="m")
                macc = wk.tile([128, 1], F32, tag="macc")
                nc.vector.affine_mul_reduce(m[:], macc[:], tau[:], dlt[:], 0.5, 0.5)
                h2 = hp.tile([128, NC_H * BL], F32, tag="h")
                pool_eng.tensor_tensor(h2[:], m[:], h[:], OP.add)
                h = h2
                if MM_BF16:
                    # chunked cast so next step's PE can start on early chunks
                    hb = hp.tile([128, NC_H * BL], mmdt, tag="hb")
                    for kc in range(NC_H):
                        sl = slice(kc * BL, (kc + 1) * BL)
                        pool_eng.tensor_copy(hb[:, sl], h[:, sl])
                else:
                    hb = h
            psl = plp.tile([1, BL], F32, tag="psl")
            for kc in range(NC_H):
                nc.tensor.matmul(
                    psl[:], wc_sb[:, kc:kc + 1], h[:, kc * BL:(kc + 1) * BL],
                    start=(kc == 0), stop=(kc == NC_H - 1),
                )
            lsb = wk.tile([1, BL], F32, tag="lsb")
            nc.scalar.activation(lsb[:], psl[:], AF.Identity, bias=bcb_sb[:])
            nc.sync.dma_start(louts[:], lsb[:])
    nc.compile()
    return nc


_CACHE = {}


def _get_nc(neg_inv_imm=None):
    key = ("nc", neg_inv_imm)
    if key not in _CACHE:
        nc = bacc.Bacc("TRN2", target_bir_lowering=False, debug=False)
        _CACHE[key] = _build(nc, neg_inv_imm=neg_inv_imm)
    return _CACHE[key]


def _host_prep(theta, Wx, Wh, b, c, sigma, q, Wc, bc):
    import ml_dtypes

    theta = np.asarray(theta, np.float32)
    weights = (1.0 / (1.0 + np.exp(-theta.astype(np.float64)))).astype(np.float32)
    wxe = (weights[:, None].astype(np.float32) * np.asarray(Wx, np.float32)).astype(np.float32)
    inv = (1.0 / (2.0 * np.asarray(sigma, np.float64) ** 2 + 1e-8)).astype(np.float32)
    neg_inv_imm = None
    if np.all(inv == inv.flat[0]):
        neg_inv_imm = float(-inv.flat[0])

    def bcast(a):  # [H,K] -> [K,128,256] at [k, p, cc*64+b] = a[cc*128+p, k]
        m = np.asarray(a, np.float32).reshape(NC_H, 128, K).transpose(2, 1, 0)  # [K,128,4]
        return np.repeat(m[..., None], BL, axis=3).reshape(K, 128, NC_H * BL).copy()

    wh = np.ascontiguousarray(np.asarray(Wh, np.float32))
    if MM_BF16:
        wh = wh.astype(ml_dtypes.bfloat16)
    d = dict(
        weights=weights,
        neg_inv_imm=neg_inv_imm,
        wxe=wxe,
        wh=wh,
        cmu=bcast(c),
        qq=bcast(q),
        bb=np.ascontiguousarray(np.asarray(b, np.float32).reshape(NC_H, 128).T),
        wc=np.ascontiguousarray(np.asarray(Wc, np.float32)[:, 0].reshape(NC_H, 128).T),
        bcb=np.asarray(bc, np.float32).reshape(1, 1),
        ident=np.eye(128, dtype=np.float32),
    )
    if neg_inv_imm is None:
        d["negi"] = bcast(-inv)
    return d


def kernel(x, theta, Wx, Wh, b, c, sigma, q, Wc, bc, _trace=False):
    x = np.ascontiguousarray(np.asarray(x, np.float32))
    prep = _host_prep(theta, Wx, Wh, b, c, sigma, q, Wc, bc)
    names = ["wxe", "wh", "cmu", "qq", "bb", "wc", "bcb", "ident"]
    if prep["neg_inv_imm"] is None:
        names.append("negi")
    shared = {k: prep[k] for k in names}
    in_maps = [dict(x=np.ascontiguousarray(x[i * BL:(i + 1) * BL]), **shared)
               for i in range(NCORES)]
    nc = _get_nc(prep["neg_inv_imm"])
    res = bass_utils.run_bass_kernel_spmd(
        nc, in_maps, core_ids=list(range(NCORES)), trace=_trace,
    )
    logits = np.concatenate([res.results[i]["logits"].reshape(BL) for i in range(NCORES)])
    out = (logits.reshape(B, 1).astype(np.float32), prep["weights"])
    if _trace:
        return out, res
    return out


# revision 23
# speedup vs baseline: 1.0873x; 1.0873x over previous
"""Trainium2 Bass kernel for the DRSAR-FL fuzzy-gated RNN.

Model (per reference):
  weights = sigmoid(theta);  xf = x * weights
  xw = einsum('btf,fh->bth', xf, Wx);  new_info = tanh(xw)
  scan over t:  z = xw_t + h @ Wh + b
                mu_k = exp(-(z-c_k)^2 * inv_k)            (K=3)
                g = sigmoid(sum(mu q)/ (sum(mu)+1e-8))
                h = (1-g) h + g new_info_t
  logits = h @ Wc + bc;  returns (logits, weights)

Sharding: data-parallel over batch B=512 -> 8 cores x 64. Weights replicated.

On-chip layout ("transposed-chunk merged"): the hidden state h[64,512] is kept
transposed as hT[128, 4*64] where partition = h%128, free = (hchunk, b).  This
makes the recurrent matmul a 16x [128k,128m]x[128,64] accumulation (contraction
on partitions) and keeps every elementwise op a full-width [128,256] tile.
xw is precomputed as xw_scan[128, T, 4, 64] so each scan step reads a dense
[128,256] slice.  x is transposed on-chip via PE-transpose; the xw matmul runs
in float32r (full-rate fp32 PE path).
"""

import numpy as np
from contextlib import ExitStack

import concourse.bacc as bacc
import concourse.bass as bass
import concourse.mybir as mybir
import concourse.tile as tile
from concourse import bass_utils

F32 = mybir.dt.float32
F32R = mybir.dt.float32r
BF16 = mybir.dt.bfloat16
AF = mybir.ActivationFunctionType
OP = mybir.AluOpType

B, T, F, H, K = 512, 128, 256, 512, 3
NCORES = 8
BL = B // NCORES          # 64 batch per core
NC_H = H // 128           # 4 h-chunks
EPS = 1e-8

# knobs
MM_BF16 = True        # recurrent h@Wh matmul in bf16
USE_DIVIDE = False    # DVE tensor_tensor divide is not a valid ISA op on TRN2
POOL_OFFLOAD = True   # run some elementwise ops on GpSimd
WARM_PE = 0           # filler matmuls hurt (serialize on PE) - keep off


def _build(nc, neg_inv_imm=None):
    mmdt = BF16 if MM_BF16 else F32
    x = nc.dram_tensor("x", [BL, T, F], F32, kind="ExternalInput")
    wxe = nc.dram_tensor("wxe", [F, H], F32, kind="ExternalInput")
    wh = nc.dram_tensor("wh", [H, H], mmdt, kind="ExternalInput")
    cmu = nc.dram_tensor("cmu", [K, 128, 256], F32, kind="ExternalInput")
    if neg_inv_imm is None:
        negi = nc.dram_tensor("negi", [K, 128, 256], F32, kind="ExternalInput")
    qq = nc.dram_tensor("qq", [K, 128, 256], F32, kind="ExternalInput")
    bb = nc.dram_tensor("bb", [128, NC_H], F32, kind="ExternalInput")
    wc = nc.dram_tensor("wc", [128, NC_H], F32, kind="ExternalInput")
    bcb = nc.dram_tensor("bcb", [1, 1], F32, kind="ExternalInput")
    ident = nc.dram_tensor("ident", [128, 128], F32, kind="ExternalInput")
    louts = nc.dram_tensor("logits", [1, BL], F32, kind="ExternalOutput")

    with tile.TileContext(nc) as tc, ExitStack() as ctx:
        const = ctx.enter_context(tc.tile_pool(name="const", bufs=1))
        wxe_sb = const.tile([128, 2, H], F32, tag="wxe")
        wh_sb = const.tile([128, NC_H, H], mmdt, tag="wh")
        cmu_sb = const.tile([128, K, 256], F32, tag="cmu")
        if neg_inv_imm is None:
            negi_sb = const.tile([128, K, 256], F32, tag="negi")
        qq_sb = const.tile([128, K, 256], F32, tag="qq")
        bb_sb = const.tile([128, NC_H], F32, tag="bb")
        wc_sb = const.tile([128, NC_H], F32, tag="wc")
        bcb_sb = const.tile([1, 1], F32, tag="bcb")
        id_sb = const.tile([128, 128], F32, tag="ident")

        nc.sync.dma_start(wxe_sb[:], wxe.rearrange("(fc p) h -> p fc h", p=128))
        nc.sync.dma_start(wh_sb[:], wh.rearrange("(kc p) h -> p kc h", p=128))
        nc.sync.dma_start(cmu_sb[:], cmu.rearrange("k p j -> p k j"))
        if neg_inv_imm is None:
            nc.sync.dma_start(negi_sb[:], negi.rearrange("k p j -> p k j"))
        nc.sync.dma_start(qq_sb[:], qq.rearrange("k p j -> p k j"))
        nc.sync.dma_start(bb_sb[:], bb[:])
        nc.sync.dma_start(wc_sb[:], wc[:])
        nc.sync.dma_start(bcb_sb[:], bcb[:])
        nc.sync.dma_start(id_sb[:], ident[:])

        xwp = ctx.enter_context(tc.tile_pool(name="xwp", bufs=1))
        xw = xwp.tile([128, T, NC_H, BL], F32, tag="xw")  # [p, t, hc, b]
        # float32r copy of Wx_eff: full-rate fp32 PE path needs f32r-rounded
        # producers for both matmul operands
        wxr = const.tile([128, 2, H], F32R, tag="wxr")
        nc.vector.tensor_copy(wxr[:], wxe_sb[:])

        # ---- precompute: transpose x, xw = xT @ Wx_eff (+b), scan layout ----
        with tc.tile_pool(name="xb", bufs=4) as xbp, \
             tc.tile_pool(name="xt", bufs=2) as xtp, \
             tc.tile_pool(name="ptr", bufs=2, space="PSUM") as ptrp, \
             tc.tile_pool(name="pxw", bufs=4, space="PSUM") as pxwp:
            for g in range(BL // 4):
                xt = xtp.tile([128, 2, 512], F32R, tag="xt")  # [f%128, fc, 4b x t]
                for bi in range(4):
                    xb = xbp.tile([128, 256], F32, tag="xb")
                    nc.sync.dma_start(xb[:], x[g * 4 + bi])
                    for fc in range(2):
                        pt = ptrp.tile([128, 128], F32, tag="pt")
                        nc.tensor.transpose(pt[:], xb[:, fc * 128:(fc + 1) * 128], id_sb[:])
                        dst = xt[:, fc, bi * 128:(bi + 1) * 128]
                        nc.vector.tensor_copy(dst, pt[:])
                for hc in range(NC_H):
                    pxw = pxwp.tile([128, 512], F32, tag="pxw")
                    for fc in range(2):
                        nc.tensor.matmul(
                            pxw[:],
                            wxr[:, fc, hc * 128:(hc + 1) * 128],
                            xt[:, fc, :],
                            start=(fc == 0), stop=(fc == 1),
                        )
                    # pxw free order is (bi, t); write into xw[p, t, hc, g*4+bi]
                    dst = xw[:, :, hc, g * 4:g * 4 + 4].rearrange("p t b -> p b t")
                    src = pxw[:].rearrange("p (b t) -> p b t", b=4)
                    nc.scalar.activation(dst, src, AF.Identity, bias=bb_sb[:, hc:hc + 1])

        # ---- scan over T ----
        with tc.tile_pool(name="psz", bufs=2, space="PSUM") as pszp, \
             tc.tile_pool(name="wk", bufs=2) as wk, \
             tc.tile_pool(name="hp", bufs=2) as hp, \
             tc.tile_pool(name="pl", bufs=1, space="PSUM") as plp, \
             tc.tile_pool(name="warm", bufs=2, space="PSUM") as warmp:
            pool_eng = nc.gpsimd if POOL_OFFLOAD else nc.vector
            h = hp.tile([128, NC_H * BL], F32, tag="h")
            nc.vector.memset(h[:], 0.0)
            hb = hp.tile([128, NC_H * BL], mmdt, tag="hb")
            nc.vector.memset(hb[:], 0.0)
            for t in range(T):
                if WARM_PE and MM_BF16:
                    # dependency-free junk matmuls: keep the PE HAM clock at
                    # 2.4 GHz across the per-step DVE/ACT phases
                    pwarm = warmp.tile([128, 512], F32, tag="pwarm")
                    for w in range(WARM_PE):
                        nc.tensor.matmul(pwarm[:], wh_sb[:, 0, 0:128],
                                         wh_sb[:, 1, :], start=(w == 0),
                                         stop=(w == WARM_PE - 1))
                psz = pszp.tile([128, NC_H * BL], F32, tag="psz")
                for mc in range(NC_H):
                    for kc in range(NC_H):
                        nc.tensor.matmul(
                            psz[:, mc * BL:(mc + 1) * BL],
                            wh_sb[:, kc, mc * 128:(mc + 1) * 128],
                            hb[:, kc * BL:(kc + 1) * BL],
                            start=(kc == 0), stop=(kc == NC_H - 1),
                        )
                xwt = xw[:, t].rearrange("p c b -> p (c b)")
                z = wk.tile([128, 256], F32, tag="z")
                nc.vector.affine_then_add(z[:], psz[:], xwt, 1.0, 0.0)

                def amul(dst, a, b_, s0=1.0, s1=0.0, tag=None):
                    acc = wk.tile([128, 1], F32, tag=f"acc_{tag}")
                    nc.vector.affine_mul_reduce(dst, acc[:], a, b_, s0, s1)

                mus = []
                for k in range(K):
                    d = wk.tile([128, 256], F32, tag=f"d{k}")
                    nc.vector.affine_then_add(d[:], cmu_sb[:, k], z[:], -1.0, 0.0)
                    mu = wk.tile([128, 256], F32, tag=f"mu{k}")
                    if neg_inv_imm is not None:
                        e = wk.tile([128, 256], F32, tag=f"e{k}")
                        amul(e[:], d[:], d[:], tag=f"e{k}")
                        nc.scalar.activation(mu[:], e[:], AF.Exp, scale=float(neg_inv_imm))
                    else:
                        u = wk.tile([128, 256], F32, tag=f"u{k}")
                        nc.vector.tensor_tensor(u[:], d[:], negi_sb[:, k], OP.mult)
                        e = wk.tile([128, 256], F32, tag=f"e{k}")
                        nc.vector.tensor_tensor(e[:], u[:], d[:], OP.mult)
                        nc.scalar.activation(mu[:], e[:], AF.Exp)
                    mus.append(mu)
                n = wk.tile([128, 256], F32, tag="n")
                amul(n[:], mus[0][:], qq_sb[:, 0], tag="n")
                t1 = wk.tile([128, 256], F32, tag="t1")
                pool_eng.tensor_tensor(t1[:], mus[1][:], qq_sb[:, 1], OP.mult)
                t2 = wk.tile([128, 256], F32, tag="t2")
                pool_eng.tensor_tensor(t2[:], mus[2][:], qq_sb[:, 2], OP.mult)
                n2 = wk.tile([128, 256], F32, tag="n2")
                nc.vector.affine_then_add(n2[:], n[:], t1[:], 1.0, 0.0)
                n3 = wk.tile([128, 256], F32, tag="n3")
                nc.vector.affine_then_add(n3[:], n2[:], t2[:], 1.0, 0.0)
                dn = wk.tile([128, 256], F32, tag="dn")
                pool_eng.tensor_tensor(dn[:], mus[0][:], mus[1][:], OP.add)
                dn3 = wk.tile([128, 256], F32, tag="dn3")
                nc.vector.affine_then_add(dn3[:], mus[2][:], dn[:], 1.0, EPS)
                r = wk.tile([128, 256], F32, tag="r")
                nc.vector.reciprocal_approx_fast(r[:], dn3[:])
                u = wk.tile([128, 256], F32, tag="u")
                amul(u[:], n3[:], r[:], tag="u")
                # sigmoid(u) = (1 + tanh(u/2)) / 2 = 0.5*tanh(u/2) + 0.5
                tau = wk.tile([128, 256], F32, tag="tau")
                nc.scalar.activation(tau[:], u[:], AF.Tanh, scale=0.5)
                nt = wk.tile([128, 256], F32, tag="nt")
                nc.scalar.activation(nt[:], xwt, AF.Tanh)
                dlt = wk.tile([128, 256], F32, tag="dlt")
                nc.vector.affine_then_add(dlt[:], h[:], nt[:], -1.0, 0.0)
                # m = g*(nt-h) with g = 0.5*tau + 0.5, fused into one DVE op
                m = wk.tile([128, 256], F32, tag="m")
                amul(m[:], tau[:], dlt[:], 0.5, 0.5, tag="m")
                h2 = hp.tile([128, NC_H * BL], F32, tag="h")
                pool_eng.tensor_tensor(h2[:], m[:], h[:], OP.add)
                h = h2
                if MM_BF16:
                    hb = hp.tile([128, NC_H * BL], mmdt, tag="hb")
                    nc.scalar.copy(hb[:], h[:])
                else:
                    hb = h
            psl = plp.tile([1, BL], F32, tag="psl")
            for kc in range(NC_H):
                nc.tensor.matmul(
                    psl[:], wc_sb[:, kc:kc + 1], h[:, kc * BL:(kc + 1) * BL],
                    start=(kc == 0), stop=(kc == NC_H - 1),
                )
            lsb = wk.tile([1, BL], F32, tag="lsb")
            nc.scalar.activation(lsb[:], psl[:], AF.Identity, bias=bcb_sb[:])
            nc.sync.dma_start(louts[:], lsb[:])
    nc.compile()
    return nc


_CACHE = {}


def _get_nc(neg_inv_imm=None):
    key = ("nc", neg_inv_imm)
    if key not in _CACHE:
        nc = bacc.Bacc("TRN2", target_bir_lowering=False, debug=False)
        _CACHE[key] = _build(nc, neg_inv_imm=neg_inv_imm)
    return _CACHE[key]


def _host_prep(theta, Wx, Wh, b, c, sigma, q, Wc, bc):
    import ml_dtypes

    theta = np.asarray(theta, np.float32)
    weights = (1.0 / (1.0 + np.exp(-theta.astype(np.float64)))).astype(np.float32)
    wxe = (weights[:, None].astype(np.float32) * np.asarray(Wx, np.float32)).astype(np.float32)
    inv = (1.0 / (2.0 * np.asarray(sigma, np.float64) ** 2 + 1e-8)).astype(np.float32)
    neg_inv_imm = None
    if np.all(inv == inv.flat[0]):
        neg_inv_imm = float(-inv.flat[0])

    def bcast(a):  # [H,K] -> [K,128,256] at [k, p, cc*64+b] = a[cc*128+p, k]
        m = np.asarray(a, np.float32).reshape(NC_H, 128, K).transpose(2, 1, 0)  # [K,128,4]
        return np.repeat(m[..., None], BL, axis=3).reshape(K, 128, NC_H * BL).copy()

    wh = np.ascontiguousarray(np.asarray(Wh, np.float32))
    if MM_BF16:
        wh = wh.astype(ml_dtypes.bfloat16)
    d = dict(
        weights=weights,
        neg_inv_imm=neg_inv_imm,
        wxe=wxe,
        wh=wh,
        cmu=bcast(c),
        qq=bcast(q),
        bb=np.ascontiguousarray(np.asarray(b, np.float32).reshape(NC_H, 128).T),
        wc=np.ascontiguousarray(np.asarray(Wc, np.float32)[:, 0].reshape(NC_H, 128).T),
        bcb=np.asarray(bc, np.float32).reshape(1, 1),
        ident=np.eye(128, dtype=np.float32),
    )
    if neg_inv_imm is None:
        d["negi"] = bcast(-inv)
    return d


def kernel(x, theta, Wx, Wh, b, c, sigma, q, Wc, bc, _trace=False):
    x = np.ascontiguousarray(np.asarray(x, np.float32))
    prep = _host_prep(theta, Wx, Wh, b, c, sigma, q, Wc, bc)
    names = ["wxe", "wh", "cmu", "qq", "bb", "wc", "bcb", "ident"]
    if prep["neg_inv_imm"] is None:
        names.append("negi")
    shared = {k: prep[k] for k in names}
    in_maps = [dict(x=np.ascontiguousarray(x[i * BL:(i + 1) * BL]), **shared)
               for i in range(NCORES)]
    nc = _get_nc(prep["neg_inv_imm"])
    res = bass_utils.run_bass_kernel_spmd(
        nc, in_maps, core_ids=list(range(NCORES)), trace=_trace,
    )
    logits = np.concatenate([res.results[i]["logits"].reshape(BL) for i in range(NCORES)])
    out = (logits.reshape(B, 1).astype(np.float32), prep["weights"])
    if _trace:
        return out, res
    return out


# revision 24
# speedup vs baseline: 1.1887x; 1.0933x over previous
"""Trainium2 Bass kernel for the DRSAR-FL fuzzy-gated RNN.

Model (per reference):
  weights = sigmoid(theta);  xf = x * weights
  xw = einsum('btf,fh->bth', xf, Wx);  new_info = tanh(xw)
  scan over t:  z = xw_t + h @ Wh + b
                mu_k = exp(-(z-c_k)^2 * inv_k)            (K=3)
                g = sigmoid(sum(mu q)/ (sum(mu)+1e-8))
                h = (1-g) h + g new_info_t
  logits = h @ Wc + bc;  returns (logits, weights)

Sharding: data-parallel over batch B=512 -> 8 cores x 64. Weights replicated.

On-chip layout ("transposed-chunk merged"): the hidden state h[64,512] is kept
transposed as hT[128, 4*64] where partition = h%128, free = (hchunk, b).  This
makes the recurrent matmul a 16x [128k,128m]x[128,64] accumulation (contraction
on partitions) and keeps every elementwise op a full-width [128,256] tile.
xw is precomputed as xw_scan[128, T, 4, 64] so each scan step reads a dense
[128,256] slice.  x is transposed on-chip via PE-transpose; the xw matmul runs
in float32r (full-rate fp32 PE path).
"""

import numpy as np
from contextlib import ExitStack

import concourse.bacc as bacc
import concourse.bass as bass
import concourse.mybir as mybir
import concourse.tile as tile
from concourse import bass_utils

F32 = mybir.dt.float32
F32R = mybir.dt.float32r
BF16 = mybir.dt.bfloat16
AF = mybir.ActivationFunctionType
OP = mybir.AluOpType

B, T, F, H, K = 512, 128, 256, 512, 3
NCORES = 8
BL = B // NCORES          # 64 batch per core
NC_H = H // 128           # 4 h-chunks
EPS = 1e-8

# knobs
MM_BF16 = True        # recurrent h@Wh matmul in bf16
USE_DIVIDE = False    # DVE tensor_tensor divide is not a valid ISA op on TRN2
POOL_OFFLOAD = True   # run some elementwise ops on GpSimd
WARM_PE = 0           # filler matmuls hurt (serialize on PE) - keep off


def _build(nc, neg_inv_imm=None):
    mmdt = BF16 if MM_BF16 else F32
    x = nc.dram_tensor("x", [BL, T, F], F32, kind="ExternalInput")
    wxe = nc.dram_tensor("wxe", [F, H], F32, kind="ExternalInput")
    wh = nc.dram_tensor("wh", [H, H], mmdt, kind="ExternalInput")
    cmu = nc.dram_tensor("cmu", [K, 128, 256], F32, kind="ExternalInput")
    if neg_inv_imm is None:
        negi = nc.dram_tensor("negi", [K, 128, 256], F32, kind="ExternalInput")
    qq = nc.dram_tensor("qq", [K, 128, 256], F32, kind="ExternalInput")
    bb = nc.dram_tensor("bb", [128, NC_H], F32, kind="ExternalInput")
    wc = nc.dram_tensor("wc", [128, NC_H], F32, kind="ExternalInput")
    bcb = nc.dram_tensor("bcb", [1, 1], F32, kind="ExternalInput")
    ident = nc.dram_tensor("ident", [128, 128], F32, kind="ExternalInput")
    louts = nc.dram_tensor("logits", [1, BL], F32, kind="ExternalOutput")

    with tile.TileContext(nc) as tc, ExitStack() as ctx:
        const = ctx.enter_context(tc.tile_pool(name="const", bufs=1))
        wxe_sb = const.tile([128, 2, H], F32, tag="wxe")
        wh_sb = const.tile([128, NC_H, H], mmdt, tag="wh")
        cmu_sb = const.tile([128, K, 256], F32, tag="cmu")
        if neg_inv_imm is None:
            negi_sb = const.tile([128, K, 256], F32, tag="negi")
        qq_sb = const.tile([128, K, 256], F32, tag="qq")
        bb_sb = const.tile([128, NC_H], F32, tag="bb")
        wc_sb = const.tile([128, NC_H], F32, tag="wc")
        bcb_sb = const.tile([1, 1], F32, tag="bcb")
        id_sb = const.tile([128, 128], F32, tag="ident")

        nc.sync.dma_start(wxe_sb[:], wxe.rearrange("(fc p) h -> p fc h", p=128))
        nc.sync.dma_start(wh_sb[:], wh.rearrange("(kc p) h -> p kc h", p=128))
        nc.sync.dma_start(cmu_sb[:], cmu.rearrange("k p j -> p k j"))
        if neg_inv_imm is None:
            nc.sync.dma_start(negi_sb[:], negi.rearrange("k p j -> p k j"))
        nc.sync.dma_start(qq_sb[:], qq.rearrange("k p j -> p k j"))
        nc.sync.dma_start(bb_sb[:], bb[:])
        nc.sync.dma_start(wc_sb[:], wc[:])
        nc.sync.dma_start(bcb_sb[:], bcb[:])
        nc.sync.dma_start(id_sb[:], ident[:])

        xwp = ctx.enter_context(tc.tile_pool(name="xwp", bufs=1))
        xw = xwp.tile([128, T, NC_H, BL], F32, tag="xw")  # [p, t, hc, b]
        # float32r copy of Wx_eff: full-rate fp32 PE path needs f32r-rounded
        # producers for both matmul operands
        wxr = const.tile([128, 2, H], F32R, tag="wxr")
        nc.vector.tensor_copy(wxr[:], wxe_sb[:])

        # ---- precompute: transpose x, xw = xT @ Wx_eff (+b), scan layout ----
        with tc.tile_pool(name="xb", bufs=4) as xbp, \
             tc.tile_pool(name="xt", bufs=2) as xtp, \
             tc.tile_pool(name="ptr", bufs=2, space="PSUM") as ptrp, \
             tc.tile_pool(name="pxw", bufs=4, space="PSUM") as pxwp:
            for g in range(BL // 4):
                xt = xtp.tile([128, 2, 512], F32R, tag="xt")  # [f%128, fc, 4b x t]
                for bi in range(4):
                    xb = xbp.tile([128, 256], F32, tag="xb")
                    nc.sync.dma_start(xb[:], x[g * 4 + bi])
                    for fc in range(2):
                        pt = ptrp.tile([128, 128], F32, tag="pt")
                        nc.tensor.transpose(pt[:], xb[:, fc * 128:(fc + 1) * 128], id_sb[:])
                        dst = xt[:, fc, bi * 128:(bi + 1) * 128]
                        nc.vector.tensor_copy(dst, pt[:])
                for hc in range(NC_H):
                    pxw = pxwp.tile([128, 512], F32, tag="pxw")
                    for fc in range(2):
                        nc.tensor.matmul(
                            pxw[:],
                            wxr[:, fc, hc * 128:(hc + 1) * 128],
                            xt[:, fc, :],
                            start=(fc == 0), stop=(fc == 1),
                        )
                    # pxw free order is (bi, t); write into xw[p, t, hc, g*4+bi]
                    dst = xw[:, :, hc, g * 4:g * 4 + 4].rearrange("p t b -> p b t")
                    src = pxw[:].rearrange("p (b t) -> p b t", b=4)
                    nc.scalar.activation(dst, src, AF.Identity, bias=bb_sb[:, hc:hc + 1])

        # ---- scan over T ----
        with tc.tile_pool(name="psz", bufs=2, space="PSUM") as pszp, \
             tc.tile_pool(name="wk", bufs=2) as wk, \
             tc.tile_pool(name="hp", bufs=2) as hp, \
             tc.tile_pool(name="pl", bufs=1, space="PSUM") as plp, \
             tc.tile_pool(name="warm", bufs=2, space="PSUM") as warmp:
            pool_eng = nc.gpsimd if POOL_OFFLOAD else nc.vector
            h = hp.tile([128, NC_H * BL], F32, tag="h")
            nc.vector.memset(h[:], 0.0)
            hb = hp.tile([128, NC_H * BL], mmdt, tag="hb")
            nc.vector.memset(hb[:], 0.0)
            for t in range(T):
                if WARM_PE and MM_BF16:
                    # dependency-free junk matmuls: keep the PE HAM clock at
                    # 2.4 GHz across the per-step DVE/ACT phases
                    pwarm = warmp.tile([128, 512], F32, tag="pwarm")
                    for w in range(WARM_PE):
                        nc.tensor.matmul(pwarm[:], wh_sb[:, 0, 0:128],
                                         wh_sb[:, 1, :], start=(w == 0),
                                         stop=(w == WARM_PE - 1))
                psz = pszp.tile([128, NC_H * BL], F32, tag="psz")
                for mc in range(NC_H):
                    for kc in range(NC_H):
                        nc.tensor.matmul(
                            psz[:, mc * BL:(mc + 1) * BL],
                            wh_sb[:, kc, mc * 128:(mc + 1) * 128],
                            hb[:, kc * BL:(kc + 1) * BL],
                            start=(kc == 0), stop=(kc == NC_H - 1),
                        )
                xwt = xw[:, t].rearrange("p c b -> p (c b)")
                z = wk.tile([128, 256], F32, tag="z")
                nc.vector.affine_then_add(z[:], psz[:], xwt, 1.0, 0.0)

                def amul(dst, a, b_, s0=1.0, s1=0.0, tag=None):
                    acc = wk.tile([128, 1], F32, tag=f"acc_{tag}")
                    nc.vector.affine_mul_reduce(dst, acc[:], a, b_, s0, s1)

                mus = []
                for k in range(K):
                    d = wk.tile([128, 256], F32, tag=f"d{k}")
                    nc.vector.affine_then_add(d[:], cmu_sb[:, k], z[:], -1.0, 0.0)
                    mu = wk.tile([128, 256], F32, tag=f"mu{k}")
                    if neg_inv_imm is not None:
                        e = wk.tile([128, 256], F32, tag=f"e{k}")
                        amul(e[:], d[:], d[:], tag=f"e{k}")
                        nc.scalar.activation(mu[:], e[:], AF.Exp, scale=float(neg_inv_imm))
                    else:
                        u = wk.tile([128, 256], F32, tag=f"u{k}")
                        nc.vector.tensor_tensor(u[:], d[:], negi_sb[:, k], OP.mult)
                        e = wk.tile([128, 256], F32, tag=f"e{k}")
                        nc.vector.tensor_tensor(e[:], u[:], d[:], OP.mult)
                        nc.scalar.activation(mu[:], e[:], AF.Exp)
                    mus.append(mu)
                n = wk.tile([128, 256], F32, tag="n")
                amul(n[:], mus[0][:], qq_sb[:, 0], tag="n")
                t1 = wk.tile([128, 256], F32, tag="t1")
                pool_eng.tensor_tensor(t1[:], mus[1][:], qq_sb[:, 1], OP.mult)
                t2 = wk.tile([128, 256], F32, tag="t2")
                pool_eng.tensor_tensor(t2[:], mus[2][:], qq_sb[:, 2], OP.mult)
                n2 = wk.tile([128, 256], F32, tag="n2")
                nc.vector.affine_then_add(n2[:], n[:], t1[:], 1.0, 0.0)
                n3 = wk.tile([128, 256], F32, tag="n3")
                nc.vector.affine_then_add(n3[:], n2[:], t2[:], 1.0, 0.0)
                dn = wk.tile([128, 256], F32, tag="dn")
                pool_eng.tensor_tensor(dn[:], mus[0][:], mus[1][:], OP.add)
                dn3 = wk.tile([128, 256], F32, tag="dn3")
                nc.vector.affine_then_add(dn3[:], mus[2][:], dn[:], 1.0, EPS)
                r = wk.tile([128, 256], F32, tag="r")
                nc.vector.reciprocal_approx_fast(r[:], dn3[:])
                u = wk.tile([128, 256], F32, tag="u")
                amul(u[:], n3[:], r[:], tag="u")
                # sigmoid(u) = (1 + tanh(u/2)) / 2 = 0.5*tanh(u/2) + 0.5
                tau = wk.tile([128, 256], F32, tag="tau")
                nc.scalar.activation(tau[:], u[:], AF.Tanh, scale=0.5)
                nt = wk.tile([128, 256], F32, tag="nt")
                nc.scalar.activation(nt[:], xwt, AF.Tanh)
                dlt = wk.tile([128, 256], F32, tag="dlt")
                nc.vector.affine_then_add(dlt[:], h[:], nt[:], -1.0, 0.0)
                # m = g*(nt-h) with g = 0.5*tau + 0.5, fused into one DVE op
                m = wk.tile([128, 256], F32, tag="m")
                amul(m[:], tau[:], dlt[:], 0.5, 0.5, tag="m")
                if MM_BF16:
                    # bf16 state for the PE first: unblocks next step's matmuls
                    # ~1.5us earlier than (pool-add -> ACT cast) did
                    hb = hp.tile([128, NC_H * BL], mmdt, tag="hb")
                    nc.vector.tensor_tensor(hb[:], m[:], h[:], OP.add)
                h2 = hp.tile([128, NC_H * BL], F32, tag="h")
                nc.vector.tensor_tensor(h2[:], m[:], h[:], OP.add)
                h = h2
                if not MM_BF16:
                    hb = h
            psl = plp.tile([1, BL], F32, tag="psl")
            for kc in range(NC_H):
                nc.tensor.matmul(
                    psl[:], wc_sb[:, kc:kc + 1], h[:, kc * BL:(kc + 1) * BL],
                    start=(kc == 0), stop=(kc == NC_H - 1),
                )
            lsb = wk.tile([1, BL], F32, tag="lsb")
            nc.scalar.activation(lsb[:], psl[:], AF.Identity, bias=bcb_sb[:])
            nc.sync.dma_start(louts[:], lsb[:])
    nc.compile()
    return nc


_CACHE = {}


def _get_nc(neg_inv_imm=None):
    key = ("nc", neg_inv_imm)
    if key not in _CACHE:
        nc = bacc.Bacc("TRN2", target_bir_lowering=False, debug=False)
        _CACHE[key] = _build(nc, neg_inv_imm=neg_inv_imm)
    return _CACHE[key]


def _host_prep(theta, Wx, Wh, b, c, sigma, q, Wc, bc):
    import ml_dtypes

    theta = np.asarray(theta, np.float32)
    weights = (1.0 / (1.0 + np.exp(-theta.astype(np.float64)))).astype(np.float32)
    wxe = (weights[:, None].astype(np.float32) * np.asarray(Wx, np.float32)).astype(np.float32)
    inv = (1.0 / (2.0 * np.asarray(sigma, np.float64) ** 2 + 1e-8)).astype(np.float32)
    neg_inv_imm = None
    if np.all(inv == inv.flat[0]):
        neg_inv_imm = float(-inv.flat[0])

    def bcast(a):  # [H,K] -> [K,128,256] at [k, p, cc*64+b] = a[cc*128+p, k]
        m = np.asarray(a, np.float32).reshape(NC_H, 128, K).transpose(2, 1, 0)  # [K,128,4]
        return np.repeat(m[..., None], BL, axis=3).reshape(K, 128, NC_H * BL).copy()

    wh = np.ascontiguousarray(np.asarray(Wh, np.float32))
    if MM_BF16:
        wh = wh.astype(ml_dtypes.bfloat16)
    d = dict(
        weights=weights,
        neg_inv_imm=neg_inv_imm,
        wxe=wxe,
        wh=wh,
        cmu=bcast(c),
        qq=bcast(q),
        bb=np.ascontiguousarray(np.asarray(b, np.float32).reshape(NC_H, 128).T),
        wc=np.ascontiguousarray(np.asarray(Wc, np.float32)[:, 0].reshape(NC_H, 128).T),
        bcb=np.asarray(bc, np.float32).reshape(1, 1),
        ident=np.eye(128, dtype=np.float32),
    )
    if neg_inv_imm is None:
        d["negi"] = bcast(-inv)
    return d


def kernel(x, theta, Wx, Wh, b, c, sigma, q, Wc, bc, _trace=False):
    x = np.ascontiguousarray(np.asarray(x, np.float32))
    prep = _host_prep(theta, Wx, Wh, b, c, sigma, q, Wc, bc)
    names = ["wxe", "wh", "cmu", "qq", "bb", "wc", "bcb", "ident"]
    if prep["neg_inv_imm"] is None:
        names.append("negi")
    shared = {k: prep[k] for k in names}
    in_maps = [dict(x=np.ascontiguousarray(x[i * BL:(i + 1) * BL]), **shared)
               for i in range(NCORES)]
    nc = _get_nc(prep["neg_inv_imm"])
    res = bass_utils.run_bass_kernel_spmd(
        nc, in_maps, core_ids=list(range(NCORES)), trace=_trace,
    )
    logits = np.concatenate([res.results[i]["logits"].reshape(BL) for i in range(NCORES)])
    out = (logits.reshape(B, 1).astype(np.float32), prep["weights"])
    if _trace:
        return out, res
    return out
